# revision 12
# baseline (speedup 1.0000x reference)
"""ChannelAttention (XCA-style cross-covariance attention) TRN2 kernel.

Shapes (hardcoded): x [8, 128, 128, 128] f32 (B, H, W, C), C=128, heads=4,
hd=32, N = H*W = 16384 tokens per sample. 8 NeuronCores, data-parallel over
batch: core i processes sample i, weights replicated, no collectives.

Algebraic reduction: attention is over channels with l2-normalization over
the full token axis, so per sample everything collapses to
  S   = X^T [X|1] Gram stats:  S = X^T X (128x128), s = X^T 1 (128)
  G   = Wq^T S Wk + qb (x) (s^T Wk + N kb) + (Wq^T s) (x) kb
  sqq = diag(Wq^T S Wq) + 2 qb*(s^T Wq) + N qb^2   (same for k with kb)
  logits_h = exp(scale_h) * rsqrt(sqq) * G * rsqrt(sqk) ; A = softmax rows
  P   = blockdiag(A)^T @ proj_w ;  Wf = Wv @ P ;  bf = v_bias @ P + proj_b
  Y   = X @ Wf + bf
Two streaming passes over X (Gram + PE-transpose via identity, then the
output GEMM) plus a small serial middle section. The reference interleaves
qkv_w columns as (head, {q,k,v}, hd); weights are permuted host-side to
[Wq|Wk|Wv] blocks with matching effective biases.
"""

import os
import sys
import types

import numpy as np

from concourse import bacc, mybir
import concourse.bass as bass
import concourse.tile as tile
from concourse.bass_utils import run_bass_kernel_spmd
from concourse.masks import make_identity

F32 = mybir.dt.float32
BF16 = mybir.dt.bfloat16

B, H, W, C = 8, 128, 128, 128
NTOK = H * W          # 16384 tokens per sample
NT = NTOK // 128      # 128 token-tiles of 128 tokens
CHUNK = 8             # token-tiles per DMA chunk
NCH = NT // CHUNK     # 16 chunks
GRP = 4               # token-tiles per PSUM bank group (4*128 = 512 f32)
HEADS, HD = 4, 32
EPS = 1.55e-05

LAST_EXEC_TIME_NS = None
_CACHED_NC = None


def _install_ntff_hook():
    """Register the axon NTFF profile hook if the image's antenv lacks it."""
    try:
        import antenv.axon_hooks  # noqa: F401
        return
    except ImportError:
        pass
    try:
        from trn_agent_boot.trn_boot import _ntff_profile_via_ctypes
        hook = _ntff_profile_via_ctypes("/opt/axon/libaxon_pjrt.so")
        mod = types.ModuleType("antenv.axon_hooks")
        mod.get_axon_ntff_profile_hook = lambda: hook
        sys.modules["antenv.axon_hooks"] = mod
    except Exception:
        pass


def build():
    nc = bacc.Bacc(None, target_bir_lowering=False)

    x_d = nc.declare_dram_parameter("x", [NTOK, C], F32, isOutput=False)
    qkvw_d = nc.declare_dram_parameter("qkv_w", [C, 3 * C], F32, isOutput=False)
    qb_d = nc.declare_dram_parameter("q_bias", [C], F32, isOutput=False)
    kb_d = nc.declare_dram_parameter("k_bias", [C], F32, isOutput=False)
    nkb_d = nc.declare_dram_parameter("n_k_bias", [C], F32, isOutput=False)
    vb_d = nc.declare_dram_parameter("v_bias", [C], F32, isOutput=False)
    esc_d = nc.declare_dram_parameter("esc_col", [C, 1], F32, isOutput=False)
    qkb_d = nc.declare_dram_parameter("qk_bias_c", [C, 2], F32, isOutput=False)
    qkbsq_d = nc.declare_dram_parameter("qk_bias_sq_n_c", [C, 2], F32,
                                        isOutput=False)
    pw_d = nc.declare_dram_parameter("proj_w", [C, C], F32, isOutput=False)
    pb_d = nc.declare_dram_parameter("proj_b", [C], F32, isOutput=False)
    out_d = nc.declare_dram_parameter("out", [NTOK, C], F32, isOutput=True)

    # token t = ch*1024 + p*8 + n -> partition p reads/writes 8 contiguous
    # rows (4 KB) per chunk DMA; the same permutation is used on the way out,
    # so it cancels.
    x_t = x_d.ap().rearrange("(ch p n) c -> ch p n c", p=128, n=CHUNK)
    out_t = out_d.ap().rearrange("(ch p n) c -> ch p n c", p=128, n=CHUNK)

    with tile.TileContext(nc) as tc:
        from contextlib import ExitStack
        with (
            tc.tile_pool(name="singles", bufs=1) as singles,
            tc.tile_pool(name="mid", bufs=1) as mid,
        ):
            mid_ctx = ExitStack()
            psum_s = mid_ctx.enter_context(
                tc.tile_pool(name="psum_s", bufs=1, space="PSUM"))
            psum_wv = mid_ctx.enter_context(
                tc.tile_pool(name="psum_wv", bufs=1, space="PSUM"))

            # ---- first x chunk DMAs go out before everything else --------
            xin_pre = []
            for ci in range(3):
                xpre = singles.tile([128, CHUNK, C], F32, tag=f"xin_pre{ci}")
                nc.sync.dma_start(xpre[:], x_t[ci])
                xin_pre.append(xpre)

            # ---- constants / weights -------------------------------------
            ident_bf = singles.tile([128, 128], BF16)
            make_identity(nc, ident_bf[:])
            ident_f32 = singles.tile([128, 128], F32)
            make_identity(nc, ident_f32[:])

            one_one = singles.tile([1, 1], F32)
            nc.vector.memset(one_one[:], 1.0)
            ones_row = singles.tile([1, C], F32)
            nc.vector.memset(ones_row[:], 1.0)
            ones_col = singles.tile([128, 1], F32)
            nc.vector.memset(ones_col[:], 1.0)
            ones_col_bf = singles.tile([128, 1], BF16)
            nc.vector.memset(ones_col_bf[:], 1.0)
            ones_row_bf = singles.tile([1, C], BF16)
            nc.vector.memset(ones_row_bf[:], 1.0)
            attn_big = mid.tile([128, 128], BF16)
            nc.gpsimd.memset(attn_big[:], 0.0)

            # ---- pass 1: Gram stats + transpose of x ---------------------
            xT_store = singles.tile([C, NTOK], BF16)
            s_ps = psum_s.tile([C, C + 1], F32)
            act_warm = singles.tile([1, 1], F32)
            nc.scalar.sqrt(act_warm[:], one_one[:])

            with (
                tc.tile_pool(name="xin", bufs=4) as xin_pool,
                tc.tile_pool(name="xbf", bufs=4) as xbf_pool,
                tc.tile_pool(name="psum_xt", bufs=5, space="PSUM") as psum_xt,
            ):
                for ch in range(NCH):
                    if ch < 3:
                        xin = xin_pre[ch]
                    else:
                        xin = xin_pool.tile([128, CHUNK, C], F32)
                        nc.sync.dma_start(xin[:], x_t[ch])
                    # cast the whole chunk to bf16 (strided dst leaves room
                    # for a ones column per tile)
                    xb = xbf_pool.tile([128, CHUNK, C + 1], BF16)
                    nc.vector.tensor_copy(xb[:, :, 0:C], xin[:])
                    nc.vector.memset(xb[:, :, C], 1.0)
                    for grp in range(CHUNK // GRP):
                        xt_ps = psum_xt.tile([C, GRP * 128], F32)
                        for k in range(GRP):
                            n = grp * GRP + k
                            g = ch * CHUNK + n
                            nc.tensor.matmul(
                                s_ps[:], lhsT=xb[:, n, 0:C], rhs=xb[:, n, :],
                                start=(g == 0), stop=(g == NT - 1))
                            nc.tensor.matmul(
                                xt_ps[:, k * 128:(k + 1) * 128],
                                lhsT=xb[:, n, 0:C], rhs=ident_bf[:],
                                start=True, stop=True)
                        dst = xT_store[:, (ch * CHUNK + grp * GRP) * 128:
                                       (ch * CHUNK + grp * GRP + GRP) * 128]
                        if grp % 2 == 0:
                            nc.scalar.copy(dst, xt_ps[:])
                        else:
                            nc.vector.tensor_copy(dst, xt_ps[:])

            # ---- middle: attention matrix -> Wf, bf ----------------------
            w_sb = singles.tile([C, 3 * C], F32)
            nc.sync.dma_start(w_sb[:], qkvw_d[:, :])
            pw_sb = singles.tile([C, C], F32)
            nc.sync.dma_start(pw_sb[:], pw_d[:, :])
            qb_row = singles.tile([1, C], F32)
            nc.sync.dma_start(qb_row[:], qb_d[None, :])
            kb_row = singles.tile([1, C], F32)
            nc.sync.dma_start(kb_row[:], kb_d[None, :])
            nkb_row = singles.tile([1, C], F32)
            nc.sync.dma_start(nkb_row[:], nkb_d[None, :])
            pb_row = singles.tile([1, C], F32)
            nc.sync.dma_start(pb_row[:], pb_d[None, :])
            esc_col = singles.tile([C, 1], F32)
            nc.sync.dma_start(esc_col[:], esc_d[:, :])
            qkb_c = singles.tile([C, 2], F32)
            nc.sync.dma_start(qkb_c[:], qkb_d[:, :])
            qkbsq_c = singles.tile([C, 2], F32)
            nc.sync.dma_start(qkbsq_c[:], qkbsq_d[:, :])
            vb_col_f = singles.tile([C, 1], F32)
            nc.sync.dma_start(vb_col_f[:], vb_d[:, None])
            vb_col = singles.tile([C, 1], BF16)
            nc.vector.tensor_copy(vb_col[:], vb_col_f[:])

            # x-independent middle piece: Wv^T (PE transpose via identity)
            wv_bf = mid.tile([C, C], BF16)
            nc.vector.tensor_copy(wv_bf[:], w_sb[:, 2 * C:3 * C])
            wvT_sb = mid.tile([C, C], BF16)
            wvT_ps = psum_wv.tile([C, C], F32, tag="wvps")
            nc.tensor.matmul(wvT_ps[:], lhsT=wv_bf[:], rhs=ident_bf[:],
                             start=True, stop=True)
            nc.vector.tensor_copy(wvT_sb[:], wvT_ps[:])


            psum_mid = mid_ctx.enter_context(
                tc.tile_pool(name="psum_mid", bufs=4, space="PSUM"))
            s_sb = mid.tile([C, C + 1], F32)
            nc.vector.tensor_copy(s_sb[:], s_ps[:])

            # SW = S @ [Wq | Wk]  (S symmetric)
            sw_ps = psum_mid.tile([C, 2 * C], F32, tag="mps")
            nc.tensor.matmul(sw_ps[:], lhsT=s_sb[:, 0:C], rhs=w_sb[:, 0:2 * C],
                             start=True, stop=True)
            sw_sb = mid.tile([C, 2 * C], F32)
            nc.vector.tensor_copy(sw_sb[:], sw_ps[:])

            # srow = s^T [Wq | Wk] (as a row, for the G rank-1 terms)
            srow_ps = psum_mid.tile([1, 2 * C], F32, tag="mps")
            nc.tensor.matmul(srow_ps[:], lhsT=s_sb[:, C:C + 1],
                             rhs=w_sb[:, 0:2 * C], start=True, stop=True)
            srow_sb = mid.tile([1, 2 * C], F32)
            nc.vector.tensor_copy(srow_sb[:], srow_ps[:])
            # and as two columns (for the sq assembly), straight off the MMs
            srow_c_ps = psum_mid.tile([C, 2], F32, tag="mps")
            nc.tensor.matmul(srow_c_ps[:, 0:1], lhsT=w_sb[:, 0:C],
                             rhs=s_sb[:, C:C + 1], start=True, stop=True)
            nc.tensor.matmul(srow_c_ps[:, 1:2], lhsT=w_sb[:, C:2 * C],
                             rhs=s_sb[:, C:C + 1], start=True, stop=True)

            # sq columns: colsum([Wq|Wk] .* SW) + 2*[qb|kb]*srow + N*[qb|kb]^2
            prod_sb = mid.tile([C, 2 * C], BF16)
            nc.vector.tensor_mul(prod_sb[:], w_sb[:, 0:2 * C], sw_sb[:])
            sq_ps = psum_mid.tile([C, 2], F32, tag="mps")
            nc.tensor.matmul(sq_ps[:, 0:1], lhsT=prod_sb[:, 0:C],
                             rhs=ones_col_bf[:], start=True, stop=True)
            nc.tensor.matmul(sq_ps[:, 1:2], lhsT=prod_sb[:, C:2 * C],
                             rhs=ones_col_bf[:], start=True, stop=True)
            sq_c = mid.tile([C, 2], F32)
            nc.vector.tensor_add(sq_c[:], sq_ps[:], qkbsq_c[:])
            t_qk = mid.tile([C, 2], F32)
            nc.vector.tensor_mul(t_qk[:], qkb_c[:], srow_c_ps[:])
            nc.vector.scalar_tensor_tensor(
                sq_c[:], t_qk[:], 2.0, sq_c[:],
                op0=mybir.AluOpType.mult, op1=mybir.AluOpType.add)

            # rqk = 1/sqrt(max(sq, EPS)) as columns; rq picks up exp(scale)
            nc.vector.tensor_scalar_max(sq_c[:], sq_c[:], EPS)
            nc.scalar.sqrt(sq_c[:], sq_c[:])
            nc.scalar.activation(act_warm[:], act_warm[:],
                                 mybir.ActivationFunctionType.Exp)
            rqk_c = mid.tile([C, 2], F32)
            nc.vector.reciprocal(rqk_c[:], sq_c[:])
            rq_col = mid.tile([C, 1], F32)
            nc.vector.tensor_mul(rq_col[:], rqk_c[:, 0:1], esc_col[:])

            # G = Wq^T S Wk + qb (x) (srow_k + N*kb) + (Wq^T s) (x) kb
            srowkn = mid.tile([1, C], F32)
            nc.vector.tensor_add(srowkn[:], srow_sb[:, C:2 * C], nkb_row[:])
            g_ps = psum_mid.tile([C, C], F32, tag="mps")
            nc.tensor.matmul(g_ps[:], lhsT=w_sb[:, 0:C], rhs=sw_sb[:, C:2 * C],
                             start=True, stop=False)
            nc.tensor.matmul(g_ps[:], lhsT=qb_row[:], rhs=srowkn[:],
                             start=False, stop=False)
            nc.tensor.matmul(g_ps[:], lhsT=srow_sb[:, 0:C], rhs=kb_row[:],
                             start=False, stop=True)

            # rk back to a row, then broadcast to all partitions
            rkr_ps = psum_mid.tile([1, C], F32, tag="mps")
            nc.tensor.matmul(rkr_ps[:], lhsT=rqk_c[:, 1:2], rhs=ident_f32[:],
                             start=True, stop=True)
            rk_row = mid.tile([1, C], F32)
            nc.vector.tensor_copy(rk_row[:], rkr_ps[:])
            rkb_ps = psum_mid.tile([C, C], F32, tag="mps")
            nc.tensor.matmul(rkb_ps[:], lhsT=ones_row[:], rhs=rk_row[:],
                             start=True, stop=True)
            rk_bc = mid.tile([C, C], F32)
            nc.vector.tensor_copy(rk_bc[:], rkb_ps[:])

            # per-head 32x32 logit blocks; softmax over rows. The 1/sum(exp)
            # is folded into proj_w rows instead of scaling the attn blocks.
            logits = mid.tile([128, 128], F32)
            nc.vector.tensor_scalar(logits[:], g_ps[:], rq_col[:, 0:1], None,
                                    op0=mybir.AluOpType.mult)
            blk = mid.tile([128, HD], F32)
            for h in range(HEADS):
                r = slice(h * HD, (h + 1) * HD)
                nc.vector.tensor_mul(blk[r, :], logits[r, r], rk_bc[r, r])

            mx = mid.tile([128, 1], F32)
            nc.vector.reduce_max(mx[:], blk[:], axis=mybir.AxisListType.X)
            nc.vector.tensor_scalar(blk[:], blk[:], mx[:, 0:1], None,
                                    op0=mybir.AluOpType.subtract)
            sumx = mid.tile([128, 1], F32)
            nc.scalar.activation(blk[:], blk[:], mybir.ActivationFunctionType.Exp,
                                 accum_out=sumx[:])
            rs = mid.tile([128, 1], F32)
            nc.vector.reciprocal(rs[:], sumx[:])
            pw_scaled = mid.tile([C, C], BF16)
            nc.vector.tensor_scalar(pw_scaled[:], pw_sb[:], rs[:, 0:1], None,
                                    op0=mybir.AluOpType.mult)
            for h in range(HEADS):
                r = slice(h * HD, (h + 1) * HD)
                nc.vector.tensor_copy(attn_big[r, r], blk[r, :])

            # P = blockdiag(exp)^T @ (pw/rowsum) ; bf = vb@P + pb ; Wf = Wv@P
            # (bias chain first so the last PE op before pass 2 is the Wf mm)
            p_ps = psum_mid.tile([C, C], F32, tag="mps")
            nc.tensor.matmul(p_ps[:], lhsT=attn_big[:], rhs=pw_scaled[:],
                             start=True, stop=True)
            p_sb = mid.tile([C, C], BF16)
            nc.vector.tensor_copy(p_sb[:], p_ps[:])

            bf_ps = psum_mid.tile([1, C], F32, tag="mps")
            nc.tensor.matmul(bf_ps[:], lhsT=vb_col[:], rhs=p_sb[:],
                             start=True, stop=True)
            bfin_row = mid.tile([1, C], F32)
            nc.vector.tensor_add(bfin_row[:], bf_ps[:], pb_row[:])
            bfin4 = mid.tile([1, GRP, C], BF16)
            nc.vector.tensor_copy(bfin4[:],
                                  bfin_row[:, None, :].to_broadcast((1, GRP, C)))
            bb_ps = psum_mid.tile([C, GRP * C], F32, tag="mps")
            nc.tensor.matmul(bb_ps[:], lhsT=ones_row_bf[:],
                             rhs=bfin4[:].rearrange("p g c -> p (g c)"),
                             start=True, stop=True)
            b_bc4 = mid.tile([C, GRP * C], F32)
            nc.vector.tensor_copy(b_bc4[:], bb_ps[:])

            wf_ps = psum_mid.tile([C, C], F32, tag="mps")
            nc.tensor.matmul(wf_ps[:], lhsT=wvT_sb[:], rhs=p_sb[:],
                             start=True, stop=True)
            wf_bf = mid.tile([C, C], BF16)
            nc.vector.tensor_copy(wf_bf[:], wf_ps[:])

            # ---- pass 2: Y = X @ Wf + bf ---------------------------------
            mid_ctx.close()
            with (
                tc.tile_pool(name="yout", bufs=8, space="SBUF") as yout_pool,
                tc.tile_pool(name="psum_y", bufs=7, space="PSUM") as psum_y,
            ):
                bfin4_flat = bfin4[:].rearrange("p g c -> p (g c)")
                gi = 0
                for ch in range(NCH):
                    for grp in range(CHUNK // GRP):
                        yout = yout_pool.tile([128, GRP, C], F32)
                        y_ps = psum_y.tile([128, GRP * C], F32)
                        pe_bias = gi % 2 == 1
                        if pe_bias:
                            nc.tensor.matmul(
                                y_ps[:], lhsT=ones_row_bf[:], rhs=bfin4_flat,
                                start=True, stop=False, skip_group_check=True)
                        for k in range(GRP):
                            g = ch * CHUNK + grp * GRP + k
                            nc.tensor.matmul(
                                y_ps[:, k * C:(k + 1) * C],
                                lhsT=xT_store[:, g * 128:(g + 1) * 128],
                                rhs=wf_bf[:], start=not pe_bias,
                                stop=(not pe_bias) or k == GRP - 1,
                                skip_group_check=True)
                        if pe_bias:
                            nc.scalar.copy(
                                yout[:].rearrange("p n c -> p (n c)"), y_ps[:])
                        else:
                            nc.vector.tensor_add(
                                yout[:].rearrange("p n c -> p (n c)"),
                                y_ps[:], b_bc4[:])
                        nc.sync.dma_start(
                            out_t[ch, :, grp * GRP:(grp + 1) * GRP, :], yout[:])
                        gi += 1

    nc.compile()
    return nc


def kernel(x, qkv_w, q_bias, v_bias, scale, proj_w, proj_b, num_heads=4):
    global _CACHED_NC, LAST_EXEC_TIME_NS
    _install_ntff_hook()
    if _CACHED_NC is None:
        _CACHED_NC = build()
    nc = _CACHED_NC

    x = np.asarray(x, dtype=np.float32)
    qkv_w = np.asarray(qkv_w, dtype=np.float32)
    q_bias = np.asarray(q_bias, dtype=np.float32)
    v_bias = np.asarray(v_bias, dtype=np.float32)
    scale = np.asarray(scale, dtype=np.float32).reshape(HEADS)
    # reference reshapes qkv to (..., heads, 3, hd): column (h, t, d) of qkv_w
    # is h*96 + t*32 + d, and bias384 = concat(q_bias, 0, v_bias) is applied
    # in that interleaved order. Permute host-side to [Wq | Wk | Wv] blocks
    # with matching effective biases (k picks up a nonzero bias).
    idx = np.concatenate([np.arange(h * 3 * HD, h * 3 * HD + HD)
                          for h in range(HEADS)])
    bias384 = np.concatenate([q_bias, np.zeros_like(q_bias), v_bias])
    w_perm = np.concatenate(
        [qkv_w[:, idx], qkv_w[:, idx + HD], qkv_w[:, idx + 2 * HD]], axis=1)
    qbe, kbe, vbe = bias384[idx], bias384[idx + HD], bias384[idx + 2 * HD]
    qkb = np.concatenate([qbe, kbe])
    shared = {
        "qkv_w": np.ascontiguousarray(w_perm),
        "q_bias": np.ascontiguousarray(qbe),
        "k_bias": np.ascontiguousarray(kbe),
        "n_k_bias": np.ascontiguousarray(np.float32(NTOK) * kbe),
        "v_bias": np.ascontiguousarray(vbe),
        "esc_col": np.ascontiguousarray(
            np.repeat(np.exp(scale), HD).reshape(C, 1)),
        "qk_bias_c": np.ascontiguousarray(np.stack([qbe, kbe], axis=1)),
        "qk_bias_sq_n_c": np.ascontiguousarray(
            np.float32(NTOK) * np.stack([qbe * qbe, kbe * kbe], axis=1)),
        "proj_w": np.ascontiguousarray(np.asarray(proj_w, dtype=np.float32)),
        "proj_b": np.ascontiguousarray(np.asarray(proj_b, dtype=np.float32)),
    }
    in_maps = [
        {"x": np.ascontiguousarray(x[i].reshape(NTOK, C)), **shared}
        for i in range(B)
    ]
    trace = bool(os.environ.get("BASS_TRACE"))
    res = run_bass_kernel_spmd(nc, in_maps, core_ids=list(range(B)), trace=trace)
    LAST_EXEC_TIME_NS = res.exec_time_ns
    return np.stack([res.results[i]["out"].reshape(H, W, C) for i in range(B)])


# revision 13
# speedup vs baseline: 1.0902x; 1.0902x over previous
"""ChannelAttention (XCA-style cross-covariance attention) TRN2 kernel.

Shapes (hardcoded): x [8, 128, 128, 128] f32 (B, H, W, C), C=128, heads=4,
hd=32, N = H*W = 16384 tokens per sample. 8 NeuronCores, data-parallel over
batch: core i processes sample i, weights replicated, no collectives.

Algebraic reduction: attention is over channels with l2-normalization over
the full token axis, so per sample everything collapses to
  S   = X^T [X|1] Gram stats:  S = X^T X (128x128), s = X^T 1 (128)
  G   = Wq^T S Wk + qb (x) (s^T Wk + N kb) + (Wq^T s) (x) kb
  sqq = diag(Wq^T S Wq) + 2 qb*(s^T Wq) + N qb^2   (same for k with kb)
  logits_h = exp(scale_h) * rsqrt(sqq) * G * rsqrt(sqk) ; A = softmax rows
  P   = blockdiag(A)^T @ proj_w ;  Wf = Wv @ P ;  bf = v_bias @ P + proj_b
  Y   = X @ Wf + bf
Two streaming passes over X (Gram + PE-transpose via identity, then the
output GEMM) plus a small serial middle section. The reference interleaves
qkv_w columns as (head, {q,k,v}, hd); weights are permuted host-side to
[Wq|Wk|Wv] blocks with matching effective biases.
"""

import os
import sys
import types

import numpy as np

from concourse import bacc, mybir
import concourse.bass as bass
import concourse.tile as tile
from concourse.bass_utils import run_bass_kernel_spmd
from concourse.masks import make_identity

F32 = mybir.dt.float32
BF16 = mybir.dt.bfloat16

B, H, W, C = 8, 128, 128, 128
NTOK = H * W          # 16384 tokens per sample
NT = NTOK // 128      # 128 token-tiles of 128 tokens
CHUNK = 8             # token-tiles per DMA chunk
NCH = NT // CHUNK     # 16 chunks
GRP = 4               # token-tiles per PSUM bank group (4*128 = 512 f32)
HEADS, HD = 4, 32
EPS = 1.55e-05

LAST_EXEC_TIME_NS = None
_CACHED_NC = None


def _install_ntff_hook():
    """Register the axon NTFF profile hook if the image's antenv lacks it."""
    try:
        import antenv.axon_hooks  # noqa: F401
        return
    except ImportError:
        pass
    try:
        from trn_agent_boot.trn_boot import _ntff_profile_via_ctypes
        hook = _ntff_profile_via_ctypes("/opt/axon/libaxon_pjrt.so")
        mod = types.ModuleType("antenv.axon_hooks")
        mod.get_axon_ntff_profile_hook = lambda: hook
        sys.modules["antenv.axon_hooks"] = mod
    except Exception:
        pass


def build():
    nc = bacc.Bacc(None, target_bir_lowering=False)

    x_d = nc.declare_dram_parameter("x", [NTOK, C], F32, isOutput=False)
    qkvw_d = nc.declare_dram_parameter("qkv_w", [C, 3 * C], F32, isOutput=False)
    qb_d = nc.declare_dram_parameter("q_bias", [C], F32, isOutput=False)
    kb_d = nc.declare_dram_parameter("k_bias", [C], F32, isOutput=False)
    nkb_d = nc.declare_dram_parameter("n_k_bias", [C], F32, isOutput=False)
    vb_d = nc.declare_dram_parameter("v_bias", [C], F32, isOutput=False)
    esc_d = nc.declare_dram_parameter("esc_col", [C, 1], F32, isOutput=False)
    qkb_d = nc.declare_dram_parameter("qk_bias_c", [C, 2], F32, isOutput=False)
    qkbsq_d = nc.declare_dram_parameter("qk_bias_sq_n_c", [C, 2], F32,
                                        isOutput=False)
    pw_d = nc.declare_dram_parameter("proj_w", [C, C], F32, isOutput=False)
    pb_d = nc.declare_dram_parameter("proj_b", [C], F32, isOutput=False)
    out_d = nc.declare_dram_parameter("out", [NTOK, C], F32, isOutput=True)

    # token t = ch*1024 + p*8 + n -> partition p reads/writes 8 contiguous
    # rows (4 KB) per chunk DMA; the same permutation is used on the way out,
    # so it cancels.
    x_t = x_d.ap().rearrange("(ch p n) c -> ch p n c", p=128, n=CHUNK)
    out_t = out_d.ap().rearrange("(ch p n) c -> ch p n c", p=128, n=CHUNK)

    with tile.TileContext(nc) as tc:
        from contextlib import ExitStack
        with (
            tc.tile_pool(name="singles", bufs=1) as singles,
            tc.tile_pool(name="mid", bufs=1) as mid,
        ):
            mid_ctx = ExitStack()
            psum_s = mid_ctx.enter_context(
                tc.tile_pool(name="psum_s", bufs=1, space="PSUM"))
            psum_wv = mid_ctx.enter_context(
                tc.tile_pool(name="psum_wv", bufs=1, space="PSUM"))

            # ---- first x chunk DMAs go out before everything else --------
            xin_pre = []
            for ci in range(3):
                xpre = singles.tile([128, CHUNK, C], F32, tag=f"xin_pre{ci}")
                nc.sync.dma_start(xpre[:], x_t[ci])
                xin_pre.append(xpre)

            # ---- constants / weights -------------------------------------
            ident_bf = singles.tile([128, 128], BF16)
            make_identity(nc, ident_bf[:])
            ident_f32 = singles.tile([128, 128], F32)
            make_identity(nc, ident_f32[:])

            one_one = singles.tile([1, 1], F32)
            nc.vector.memset(one_one[:], 1.0)
            ones_row = singles.tile([1, C], F32)
            nc.vector.memset(ones_row[:], 1.0)
            ones_col = singles.tile([128, 1], F32)
            nc.vector.memset(ones_col[:], 1.0)
            ones_col_bf = singles.tile([128, 1], BF16)
            nc.vector.memset(ones_col_bf[:], 1.0)
            ones_row_bf = singles.tile([1, C], BF16)
            nc.vector.memset(ones_row_bf[:], 1.0)
            attn_big = mid.tile([128, 128], BF16)
            nc.gpsimd.memset(attn_big[:], 0.0)

            # ---- pass 1: Gram stats + transpose of x ---------------------
            xT_store = singles.tile([C, NTOK], BF16)
            s_ps = psum_s.tile([C, C + 1], F32)
            act_warm = singles.tile([1, 1], F32)
            nc.scalar.sqrt(act_warm[:], one_one[:])

            with (
                tc.tile_pool(name="xin", bufs=4) as xin_pool,
                tc.tile_pool(name="xbf", bufs=4) as xbf_pool,
                tc.tile_pool(name="psum_xt", bufs=5, space="PSUM") as psum_xt,
            ):
                for ch in range(NCH):
                    if ch < 3:
                        xin = xin_pre[ch]
                    else:
                        xin = xin_pool.tile([128, CHUNK, C], F32)
                        nc.sync.dma_start(xin[:], x_t[ch])
                    # cast the whole chunk to bf16 (strided dst leaves room
                    # for a ones column per tile)
                    xb = xbf_pool.tile([128, CHUNK, C + 1], BF16)
                    nc.vector.tensor_copy(xb[:, :, 0:C], xin[:])
                    nc.vector.memset(xb[:, :, C], 1.0)
                    for grp in range(CHUNK // GRP):
                        xt_ps = psum_xt.tile([C, GRP * 128], F32)
                        for k in range(GRP):
                            n = grp * GRP + k
                            g = ch * CHUNK + n
                            nc.tensor.matmul(
                                s_ps[:], lhsT=xb[:, n, 0:C], rhs=xb[:, n, :],
                                start=(g == 0), stop=(g == NT - 1))
                            nc.tensor.matmul(
                                xt_ps[:, k * 128:(k + 1) * 128],
                                lhsT=xb[:, n, 0:C], rhs=ident_bf[:],
                                start=True, stop=True)
                        dst = xT_store[:, (ch * CHUNK + grp * GRP) * 128:
                                       (ch * CHUNK + grp * GRP + GRP) * 128]
                        if grp % 2 == 0:
                            nc.scalar.copy(dst, xt_ps[:])
                        else:
                            nc.vector.tensor_copy(dst, xt_ps[:])

            # ---- middle: attention matrix -> Wf, bf ----------------------
            w_sb = singles.tile([C, 3 * C], F32)
            nc.sync.dma_start(w_sb[:], qkvw_d[:, :])
            pw_sb = singles.tile([C, C], F32)
            nc.sync.dma_start(pw_sb[:], pw_d[:, :])
            qb_row = singles.tile([1, C], F32)
            nc.sync.dma_start(qb_row[:], qb_d[None, :])
            kb_row = singles.tile([1, C], F32)
            nc.sync.dma_start(kb_row[:], kb_d[None, :])
            nkb_row = singles.tile([1, C], F32)
            nc.sync.dma_start(nkb_row[:], nkb_d[None, :])
            pb_row = singles.tile([1, C], F32)
            nc.sync.dma_start(pb_row[:], pb_d[None, :])
            esc_col = singles.tile([C, 1], F32)
            nc.sync.dma_start(esc_col[:], esc_d[:, :])
            qkb_c = singles.tile([C, 2], F32)
            nc.sync.dma_start(qkb_c[:], qkb_d[:, :])
            qkbsq_c = singles.tile([C, 2], F32)
            nc.sync.dma_start(qkbsq_c[:], qkbsq_d[:, :])
            vb_col_f = singles.tile([C, 1], F32)
            nc.sync.dma_start(vb_col_f[:], vb_d[:, None])
            vb_col = singles.tile([C, 1], BF16)
            nc.vector.tensor_copy(vb_col[:], vb_col_f[:])

            # x-independent middle piece: Wv^T (PE transpose via identity)
            wv_bf = mid.tile([C, C], BF16)
            nc.vector.tensor_copy(wv_bf[:], w_sb[:, 2 * C:3 * C])
            wvT_sb = mid.tile([C, C], BF16)
            wvT_ps = psum_wv.tile([C, C], F32, tag="wvps")
            nc.tensor.matmul(wvT_ps[:], lhsT=wv_bf[:], rhs=ident_bf[:],
                             start=True, stop=True)
            nc.vector.tensor_copy(wvT_sb[:], wvT_ps[:])


            psum_mid = mid_ctx.enter_context(
                tc.tile_pool(name="psum_mid", bufs=4, space="PSUM"))
            s_sb = mid.tile([C, C + 1], F32)
            nc.vector.tensor_copy(s_sb[:], s_ps[:])

            # SW = S @ [Wq | Wk]  (S symmetric)
            sw_ps = psum_mid.tile([C, 2 * C], F32, tag="mps")
            nc.tensor.matmul(sw_ps[:], lhsT=s_sb[:, 0:C], rhs=w_sb[:, 0:2 * C],
                             start=True, stop=True)
            sw_sb = mid.tile([C, 2 * C], F32)
            nc.vector.tensor_copy(sw_sb[:], sw_ps[:])

            # srow = s^T [Wq | Wk] (as a row, for the G rank-1 terms)
            srow_ps = psum_mid.tile([1, 2 * C], F32, tag="mps")
            nc.tensor.matmul(srow_ps[:], lhsT=s_sb[:, C:C + 1],
                             rhs=w_sb[:, 0:2 * C], start=True, stop=True)
            srow_sb = mid.tile([1, 2 * C], F32)
            nc.vector.tensor_copy(srow_sb[:], srow_ps[:])
            # and as two columns (for the sq assembly), straight off the MMs
            srow_c_ps = psum_mid.tile([C, 2], F32, tag="mps")
            nc.tensor.matmul(srow_c_ps[:, 0:1], lhsT=w_sb[:, 0:C],
                             rhs=s_sb[:, C:C + 1], start=True, stop=True)
            nc.tensor.matmul(srow_c_ps[:, 1:2], lhsT=w_sb[:, C:2 * C],
                             rhs=s_sb[:, C:C + 1], start=True, stop=True)

            # sq columns: colsum([Wq|Wk] .* SW) + 2*[qb|kb]*srow + N*[qb|kb]^2
            prod_sb = mid.tile([C, 2 * C], BF16)
            nc.vector.tensor_mul(prod_sb[:], w_sb[:, 0:2 * C], sw_sb[:])
            sq_ps = psum_mid.tile([C, 2], F32, tag="mps")
            nc.tensor.matmul(sq_ps[:, 0:1], lhsT=prod_sb[:, 0:C],
                             rhs=ones_col_bf[:], start=True, stop=True)
            nc.tensor.matmul(sq_ps[:, 1:2], lhsT=prod_sb[:, C:2 * C],
                             rhs=ones_col_bf[:], start=True, stop=True)
            sq_c = mid.tile([C, 2], F32)
            nc.vector.tensor_add(sq_c[:], sq_ps[:], qkbsq_c[:])
            t_qk = mid.tile([C, 2], F32)
            nc.vector.tensor_mul(t_qk[:], qkb_c[:], srow_c_ps[:])
            nc.vector.scalar_tensor_tensor(
                sq_c[:], t_qk[:], 2.0, sq_c[:],
                op0=mybir.AluOpType.mult, op1=mybir.AluOpType.add)

            # rqk = 1/sqrt(max(sq, EPS)) as columns; rq picks up exp(scale)
            nc.vector.tensor_scalar_max(sq_c[:], sq_c[:], EPS)
            nc.scalar.sqrt(sq_c[:], sq_c[:])
            nc.scalar.activation(act_warm[:], act_warm[:],
                                 mybir.ActivationFunctionType.Exp)
            rqk_c = mid.tile([C, 2], F32)
            nc.vector.reciprocal(rqk_c[:], sq_c[:])
            rq_col = mid.tile([C, 1], F32)
            nc.vector.tensor_mul(rq_col[:], rqk_c[:, 0:1], esc_col[:])

            # G = Wq^T S Wk + qb (x) (srow_k + N*kb) + (Wq^T s) (x) kb
            srowkn = mid.tile([1, C], F32)
            nc.vector.tensor_add(srowkn[:], srow_sb[:, C:2 * C], nkb_row[:])
            g_ps = psum_mid.tile([C, C], F32, tag="mps")
            nc.tensor.matmul(g_ps[:], lhsT=w_sb[:, 0:C], rhs=sw_sb[:, C:2 * C],
                             start=True, stop=False)
            nc.tensor.matmul(g_ps[:], lhsT=qb_row[:], rhs=srowkn[:],
                             start=False, stop=False)
            nc.tensor.matmul(g_ps[:], lhsT=srow_sb[:, 0:C], rhs=kb_row[:],
                             start=False, stop=True)

            # rk back to a row, then broadcast to all partitions
            rkr_ps = psum_mid.tile([1, C], F32, tag="mps")
            nc.tensor.matmul(rkr_ps[:], lhsT=rqk_c[:, 1:2], rhs=ident_f32[:],
                             start=True, stop=True)
            rk_row = mid.tile([1, C], F32)
            nc.vector.tensor_copy(rk_row[:], rkr_ps[:])
            rkb_ps = psum_mid.tile([C, C], F32, tag="mps")
            nc.tensor.matmul(rkb_ps[:], lhsT=ones_row[:], rhs=rk_row[:],
                             start=True, stop=True)
            rk_bc = mid.tile([C, C], F32)
            nc.vector.tensor_copy(rk_bc[:], rkb_ps[:])

            # per-head 32x32 logit blocks; softmax over rows. The 1/sum(exp)
            # is folded into proj_w rows instead of scaling the attn blocks.
            logits = mid.tile([128, 128], F32)
            nc.vector.tensor_scalar(logits[:], g_ps[:], rq_col[:, 0:1], None,
                                    op0=mybir.AluOpType.mult)
            blk = mid.tile([128, HD], F32)
            for h in range(HEADS):
                r = slice(h * HD, (h + 1) * HD)
                nc.vector.tensor_mul(blk[r, :], logits[r, r], rk_bc[r, r])

            mx = mid.tile([128, 1], F32)
            nc.vector.reduce_max(mx[:], blk[:], axis=mybir.AxisListType.X)
            nc.vector.tensor_scalar(blk[:], blk[:], mx[:, 0:1], None,
                                    op0=mybir.AluOpType.subtract)
            sumx = mid.tile([128, 1], F32)
            nc.scalar.activation(blk[:], blk[:], mybir.ActivationFunctionType.Exp,
                                 accum_out=sumx[:])
            rs = mid.tile([128, 1], F32)
            nc.vector.reciprocal(rs[:], sumx[:])
            pw_scaled = mid.tile([C, C], BF16)
            nc.vector.tensor_scalar(pw_scaled[:], pw_sb[:], rs[:, 0:1], None,
                                    op0=mybir.AluOpType.mult)
            for h in range(HEADS):
                r = slice(h * HD, (h + 1) * HD)
                nc.vector.tensor_copy(attn_big[r, r], blk[r, :])

            # P = blockdiag(exp)^T @ (pw/rowsum) ; bf = vb@P + pb ; Wf = Wv@P
            # (bias chain first so the last PE op before pass 2 is the Wf mm)
            p_ps = psum_mid.tile([C, C], F32, tag="mps")
            nc.tensor.matmul(p_ps[:], lhsT=attn_big[:], rhs=pw_scaled[:],
                             start=True, stop=True)
            p_sb = mid.tile([C, C], BF16)
            nc.vector.tensor_copy(p_sb[:], p_ps[:])

            bf_ps = psum_mid.tile([1, C], F32, tag="mps")
            nc.tensor.matmul(bf_ps[:], lhsT=vb_col[:], rhs=p_sb[:],
                             start=True, stop=True)
            bfin_row = mid.tile([1, C], F32)
            nc.vector.tensor_add(bfin_row[:], bf_ps[:], pb_row[:])
            bfin4 = mid.tile([1, GRP, C], BF16)
            nc.vector.tensor_copy(bfin4[:],
                                  bfin_row[:, None, :].to_broadcast((1, GRP, C)))
            bb_ps = psum_mid.tile([C, GRP * C], F32, tag="mps")
            nc.tensor.matmul(bb_ps[:], lhsT=ones_row_bf[:],
                             rhs=bfin4[:].rearrange("p g c -> p (g c)"),
                             start=True, stop=True)
            b_bc4 = mid.tile([C, GRP * C], F32)
            nc.vector.tensor_copy(b_bc4[:], bb_ps[:])

            wf_ps = psum_mid.tile([C, C], F32, tag="mps")
            nc.tensor.matmul(wf_ps[:], lhsT=wvT_sb[:], rhs=p_sb[:],
                             start=True, stop=True)
            wf_bf = mid.tile([C, C], BF16)
            nc.vector.tensor_copy(wf_bf[:], wf_ps[:])

            # ---- pass 2: Y = X @ Wf + bf ---------------------------------
            mid_ctx.close()
            with (
                tc.tile_pool(name="yout", bufs=8, space="SBUF") as yout_pool,
                tc.tile_pool(name="psum_y", bufs=7, space="PSUM") as psum_y,
            ):
                for ch in range(NCH):
                    for grp in range(CHUNK // GRP):
                        yout = yout_pool.tile([128, GRP, C], F32)
                        y_ps = psum_y.tile([128, GRP * C], F32)
                        for k in range(GRP):
                            g = ch * CHUNK + grp * GRP + k
                            nc.tensor.matmul(
                                y_ps[:, k * C:(k + 1) * C],
                                lhsT=xT_store[:, g * 128:(g + 1) * 128],
                                rhs=wf_bf[:], start=True, stop=True)
                        nc.vector.tensor_add(
                            yout[:].rearrange("p n c -> p (n c)"),
                            y_ps[:], b_bc4[:])
                        nc.sync.dma_start(
                            out_t[ch, :, grp * GRP:(grp + 1) * GRP, :], yout[:])

    nc.compile()
    return nc


def kernel(x, qkv_w, q_bias, v_bias, scale, proj_w, proj_b, num_heads=4):
    global _CACHED_NC, LAST_EXEC_TIME_NS
    _install_ntff_hook()
    if _CACHED_NC is None:
        _CACHED_NC = build()
    nc = _CACHED_NC

    x = np.asarray(x, dtype=np.float32)
    qkv_w = np.asarray(qkv_w, dtype=np.float32)
    q_bias = np.asarray(q_bias, dtype=np.float32)
    v_bias = np.asarray(v_bias, dtype=np.float32)
    scale = np.asarray(scale, dtype=np.float32).reshape(HEADS)
    # reference reshapes qkv to (..., heads, 3, hd): column (h, t, d) of qkv_w
    # is h*96 + t*32 + d, and bias384 = concat(q_bias, 0, v_bias) is applied
    # in that interleaved order. Permute host-side to [Wq | Wk | Wv] blocks
    # with matching effective biases (k picks up a nonzero bias).
    idx = np.concatenate([np.arange(h * 3 * HD, h * 3 * HD + HD)
                          for h in range(HEADS)])
    bias384 = np.concatenate([q_bias, np.zeros_like(q_bias), v_bias])
    w_perm = np.concatenate(
        [qkv_w[:, idx], qkv_w[:, idx + HD], qkv_w[:, idx + 2 * HD]], axis=1)
    qbe, kbe, vbe = bias384[idx], bias384[idx + HD], bias384[idx + 2 * HD]
    qkb = np.concatenate([qbe, kbe])
    shared = {
        "qkv_w": np.ascontiguousarray(w_perm),
        "q_bias": np.ascontiguousarray(qbe),
        "k_bias": np.ascontiguousarray(kbe),
        "n_k_bias": np.ascontiguousarray(np.float32(NTOK) * kbe),
        "v_bias": np.ascontiguousarray(vbe),
        "esc_col": np.ascontiguousarray(
            np.repeat(np.exp(scale), HD).reshape(C, 1)),
        "qk_bias_c": np.ascontiguousarray(np.stack([qbe, kbe], axis=1)),
        "qk_bias_sq_n_c": np.ascontiguousarray(
            np.float32(NTOK) * np.stack([qbe * qbe, kbe * kbe], axis=1)),
        "proj_w": np.ascontiguousarray(np.asarray(proj_w, dtype=np.float32)),
        "proj_b": np.ascontiguousarray(np.asarray(proj_b, dtype=np.float32)),
    }
    in_maps = [
        {"x": np.ascontiguousarray(x[i].reshape(NTOK, C)), **shared}
        for i in range(B)
    ]
    trace = bool(os.environ.get("BASS_TRACE"))
    res = run_bass_kernel_spmd(nc, in_maps, core_ids=list(range(B)), trace=trace)
    LAST_EXEC_TIME_NS = res.exec_time_ns
    return np.stack([res.results[i]["out"].reshape(H, W, C) for i in range(B)])
